# revision 2
# baseline (speedup 1.0000x reference)
"""Trainium2 Bass kernel for DTDRLinear: y = hadamard(x) @ (Q*s)^T + bias.

Strategy (8 NeuronCores, data-parallel over tokens):
  - Each core gets 1024 tokens; x shard passed transposed (in-major) from host.
  - FWHT factorized H_4096 = H_32(outer, in//128) (x) H_128(inner, in%128):
      * outer 5 butterfly stages on DVE along the free dim of xT (f32),
      * inner H128 applied on the PE as a matmul with a constant +-1 matrix.
  - Weights streamed as Q^T (host layout permute), dequant cast int32->bf16
    in-flight by SWDGE DMA. Matmul in bf16 (Q exact in bf16), f32 PSUM accum.
  - s/64 scale and bias applied at PSUM eviction on DVE (broadcast rows).
"""

import math

import numpy as np
import ml_dtypes

import concourse.bacc as bacc
import concourse.bass as bass
import concourse.mybir as mybir
import concourse.tile as tile
from concourse import bass_utils

TOKENS = 8192
IN = 4096
OUT = 4096
NCORES = 8
T_LOC = TOKENS // NCORES  # 1024 tokens per core

F32 = mybir.dt.float32
BF16 = mybir.dt.bfloat16
I32 = mybir.dt.int32

# phase-1 token chunking for butterfly buffers
CHUNK = 128
NCHUNK = T_LOC // CHUNK
# phase-2 out_features supers (U k-slices resident per super)
OSUP = 1024
NSUP = OUT // OSUP

_cache = {}


def _build_nc():
    nc = bacc.Bacc(
        "TRN2",
        target_bir_lowering=False,
        debug=False,
        enable_asserts=False,
        num_devices=NCORES,
        num_swdge_queues=4,
    )
    xT = nc.dram_tensor("xT", [IN, T_LOC], F32, kind="ExternalInput").ap()
    qT = nc.dram_tensor("qT", [IN, OUT], I32, kind="ExternalInput").ap()
    h128 = nc.dram_tensor("h128", [128, 128], BF16, kind="ExternalInput").ap()
    s_in = nc.dram_tensor("s_in", [OUT], F32, kind="ExternalInput").ap()
    b_in = nc.dram_tensor("b_in", [OUT], F32, kind="ExternalInput").ap()
    y = nc.dram_tensor("y", [T_LOC, OUT], F32, kind="ExternalOutput").ap()

    KT = IN // 128  # 32 k-tiles
    ST = 32  # outer hadamard size (in // 128)

    xT_v = xT.rearrange("(s p) t -> p s t", p=128)  # [128, 32, T_LOC]
    qT_v = qT.rearrange("(k p) o -> k p o", p=128)  # [32, 128, OUT]

    with tile.TileContext(nc) as tc:
        with (
            tc.tile_pool(name="persist", bufs=1) as persist,
            tc.tile_pool(name="consts", bufs=1) as consts,
        ):
            # persistent transposed+hadamard'd activations, [in%128, (in//128, tok)]
            xhT = persist.tile([128, ST * T_LOC], BF16)
            h128_sb = consts.tile([128, 128], BF16)
            nc.sync.dma_start(out=h128_sb, in_=h128)

            # ---------------- Phase 1: FWHT ----------------
            with (
                tc.tile_pool(name="fw", bufs=2) as fw,
                tc.tile_pool(name="fwp", bufs=1) as fwp,
                tc.tile_pool(name="fpsum", bufs=2, space="PSUM") as fpsum,
            ):
                FREE = ST * CHUNK  # free extent per chunk
                for c in range(NCHUNK):
                    A = fw.tile([128, FREE], F32, tag="A")
                    B32 = fwp.tile([128, FREE], F32, tag="B32")
                    B16 = fw.tile([128, FREE], BF16, tag="B16")
                    nc.sync.dma_start(
                        out=A, in_=xT_v[:, :, c * CHUNK : (c + 1) * CHUNK]
                    )
                    # 5 outer butterfly stages over s = in//128 (f32),
                    # ping-pong A <-> B32, last stage writes bf16 into B16
                    bufs = [A, B32, A, B32, B16]
                    src = A
                    for stage, h in enumerate((1, 2, 4, 8, 16)):
                        dst = bufs[stage]
                        if stage == 0:
                            dst = B32
                        elif stage == 4:
                            dst = B16
                        else:
                            dst = A if (stage % 2 == 1) else B32
                        run = h * CHUNK
                        for g in range(ST // (2 * h)):
                            base = g * 2 * run
                            l = slice(base, base + run)
                            r = slice(base + run, base + 2 * run)
                            nc.vector.tensor_add(dst[:, l], src[:, l], src[:, r])
                            nc.vector.tensor_sub(dst[:, r], src[:, l], src[:, r])
                        src = dst
                    # inner H128 fold on PE; evict to xhT (strided dst)
                    xh_v = xhT.rearrange("p (s t) -> p s t", t=T_LOC)
                    SPF = 512 // CHUNK  # s-blocks covered per 512-wide matmul
                    for f in range(FREE // 512):
                        fps = fpsum.tile([128, 512], F32, tag="fps")
                        nc.tensor.matmul(
                            fps,
                            lhsT=h128_sb,
                            rhs=B16[:, f * 512 : (f + 1) * 512],
                            start=True,
                            stop=True,
                        )
                        dst = xh_v[
                            :,
                            f * SPF : (f + 1) * SPF,
                            c * CHUNK : (c + 1) * CHUNK,
                        ]
                        nc.scalar.copy(dst, fps)

            # ---------------- Phase 2: main matmul ----------------
            with (
                tc.tile_pool(name="upool", bufs=1) as upool,
                tc.tile_pool(name="sb_pool", bufs=1) as sb_pool,
                tc.tile_pool(name="ypool", bufs=4) as ypool,
                tc.tile_pool(name="mpsum", bufs=2, space="PSUM") as mpsum,
            ):
                # broadcast scale (s/64) and bias rows across partitions
                s_rep = sb_pool.tile([128, OUT], F32)
                b_rep = sb_pool.tile([128, OUT], F32)
                s_bcast = bass.AP(
                    tensor=s_in.tensor, offset=0, ap=[[0, 128], [1, OUT]]
                )
                b_bcast = bass.AP(
                    tensor=b_in.tensor, offset=0, ap=[[0, 128], [1, OUT]]
                )
                nc.gpsimd.dma_start(out=s_rep, in_=s_bcast)
                nc.gpsimd.dma_start(out=b_rep, in_=b_bcast)
                nc.vector.tensor_scalar_mul(s_rep, s_rep, 1.0 / 64.0)

                NB = OSUP // 512  # out 512-blocks per super
                for sup in range(NSUP):
                    osl = slice(sup * OSUP, (sup + 1) * OSUP)
                    # dequantized weight k-slices for this super (DMA casts)
                    U = []
                    for k in range(KT):
                        u = upool.tile([128, OSUP], BF16, tag=f"u{k}", name=f"u{k}")
                        nc.gpsimd.dma_start(out=u, in_=qT_v[k, :, osl])
                        U.append(u)
                    for t in range(T_LOC // 128):
                        pss = [
                            mpsum.tile([128, 512], F32, tag=f"ps{nb}", name=f"ps{nb}")
                            for nb in range(NB)
                        ]
                        for k in range(KT):
                            lhs = xhT[:, k * T_LOC + t * 128 : k * T_LOC + (t + 1) * 128]
                            for nb in range(NB):
                                nc.tensor.matmul(
                                    pss[nb],
                                    lhsT=lhs,
                                    rhs=U[k][:, nb * 512 : (nb + 1) * 512],
                                    start=(k == 0),
                                    stop=(k == KT - 1),
                                )
                        for nb in range(NB):
                            ob = sup * OSUP + nb * 512
                            ysb = ypool.tile([128, 512], F32, tag="ysb", name="ysb")
                            nc.vector.tensor_tensor(
                                ysb, pss[nb], s_rep[:, ob : ob + 512],
                                op=mybir.AluOpType.mult,
                            )
                            nc.vector.tensor_tensor(
                                ysb, ysb, b_rep[:, ob : ob + 512],
                                op=mybir.AluOpType.add,
                            )
                            nc.sync.dma_start(
                                out=y[t * 128 : (t + 1) * 128, ob : ob + 512],
                                in_=ysb,
                            )

    nc.compile()
    return nc


def _get_nc():
    if "nc" not in _cache:
        _cache["nc"] = _build_nc()
    return _cache["nc"]


def _h128_pm1():
    h = np.eye(128, dtype=np.float64)
    n = 128
    hh = 1
    a = h
    while hh < n:
        a = a.reshape(-1, n // (2 * hh), 2, hh, n)
        l = a[:, :, 0].copy()
        r = a[:, :, 1].copy()
        a[:, :, 0] = l + r
        a[:, :, 1] = l - r
        a = a.reshape(-1, n)
        hh *= 2
    # a = H128 (unnormalized, symmetric +-1)
    return a.astype(ml_dtypes.bfloat16)


def kernel(x, Q_tilde, s_tilde, bias):
    nc = _get_nc()
    h128 = _h128_pm1()
    qT = np.ascontiguousarray(Q_tilde.T).astype(np.int32)
    s_flat = np.ascontiguousarray(s_tilde.reshape(-1)).astype(np.float32)
    b_flat = np.ascontiguousarray(bias.reshape(-1)).astype(np.float32)
    in_maps = []
    for c in range(NCORES):
        xTc = np.ascontiguousarray(
            x[c * T_LOC : (c + 1) * T_LOC, :].T
        ).astype(np.float32)
        in_maps.append(
            {"xT": xTc, "qT": qT, "h128": h128, "s_in": s_flat, "b_in": b_flat}
        )
    res = bass_utils.run_bass_kernel_spmd(nc, in_maps, list(range(NCORES)))
    y = np.concatenate([res.results[c]["y"] for c in range(NCORES)], axis=0)
    return y.astype(np.float32)


# revision 5
# speedup vs baseline: 1.1054x; 1.1054x over previous
"""Trainium2 Bass kernel for DTDRLinear: y = hadamard(x) @ (Q*s)^T + bias.

Strategy (8 NeuronCores, data-parallel over tokens, 1024 tok/core):
  - x shard passed transposed (in-major) from host; full Q^T per core.
  - FWHT factorized H_4096 = H_32(outer, on in//128) (x) H_128(inner, in%128):
      * outer 5 butterfly stages on DVE along the free dim (fp16, 2x mode),
      * inner H128 (pre-scaled by 1/64 so the result is the normalized FWHT)
        applied on the PE as a matmul with a constant fp16 matrix.
  - Weights dequantized int32->fp16 on device (split between SWDGE cast-DMA
    and HWDGE load + gpsimd tensor_copy); Q fits fp16 exactly.
  - Main matmul fp16 x fp16 -> f32 PSUM; per-column scale s and bias applied
    at PSUM eviction on DVE against partition-broadcast rows.
  - Phase interleave: chunk c's butterflies+fold feed the super-0 main
    matmuls for token tile c, so the PE never waits for the whole FWHT.
"""

import numpy as np
import ml_dtypes

import concourse.bacc as bacc
import concourse.bass as bass
import concourse.mybir as mybir
import concourse.tile as tile
from concourse import bass_utils

TOKENS = 8192
IN = 4096
OUT = 4096
NCORES = 8
T_LOC = TOKENS // NCORES  # 1024

F32 = mybir.dt.float32
F16 = mybir.dt.float16
I32 = mybir.dt.int32

CHUNK = 128               # tokens per butterfly chunk == token tile
NCHUNK = T_LOC // CHUNK   # 8
OSUP = 1024               # out-features per resident weight super
NSUP = OUT // OSUP        # 4
KT = IN // 128            # 32 contraction tiles
ST = 32                   # outer hadamard size (in // 128)
NB = OSUP // 512          # 2 psum blocks per super

_cache = {}


def _build_nc():
    nc = bacc.Bacc(
        "TRN2",
        target_bir_lowering=False,
        debug=False,
        enable_asserts=False,
        num_devices=NCORES,
        num_swdge_queues=4,
    )
    xT = nc.dram_tensor("xT", [IN, T_LOC], F32, kind="ExternalInput").ap()
    qT = nc.dram_tensor("qT", [IN, OUT], I32, kind="ExternalInput").ap()
    h128 = nc.dram_tensor("h128", [128, 128], F16, kind="ExternalInput").ap()
    s_in = nc.dram_tensor("s_in", [OUT], F32, kind="ExternalInput").ap()
    b_in = nc.dram_tensor("b_in", [OUT], F32, kind="ExternalInput").ap()
    y = nc.dram_tensor("y", [T_LOC, OUT], F32, kind="ExternalOutput").ap()

    xT_v = xT.rearrange("(s p) t -> p s t", p=128)  # [128, 32, T_LOC]
    qT_v = qT.rearrange("(k p) o -> k p o", p=128)  # [32, 128, OUT]

    with tile.TileContext(nc) as tc:
        with (
            tc.tile_pool(name="persist", bufs=1) as persist,
            tc.tile_pool(name="consts", bufs=1) as consts,
            tc.tile_pool(name="fw", bufs=2) as fw,
            tc.tile_pool(name="upool", bufs=1) as upool,
            tc.tile_pool(name="ustage", bufs=2) as ustage,
            tc.tile_pool(name="sbp", bufs=1) as sbp,
            tc.tile_pool(name="ypool", bufs=2) as ypool,
            tc.tile_pool(name="fpsum", bufs=2, space="PSUM") as fpsum,
            tc.tile_pool(name="mpsum", bufs=2, space="PSUM") as mpsum,
        ):
            xhT = persist.tile([128, ST * T_LOC], F16)
            xh_v = xhT.rearrange("p (s t) -> p s t", t=T_LOC)
            h128_sb = consts.tile([128, 128], F16)
            nc.sync.dma_start(out=h128_sb, in_=h128)

            hw_engines = [nc.sync, nc.scalar]

            def load_super(sup):
                """scale/bias broadcast rows + dequantized U k-slices."""
                osl = slice(sup * OSUP, (sup + 1) * OSUP)
                s_rep = sbp.tile([128, OSUP], F32, tag="s_rep", name="s_rep")
                b_rep = sbp.tile([128, OSUP], F32, tag="b_rep", name="b_rep")
                nc.gpsimd.dma_start(
                    out=s_rep,
                    in_=bass.AP(
                        tensor=s_in.tensor, offset=sup * OSUP, ap=[[0, 128], [1, OSUP]]
                    ),
                )
                nc.gpsimd.dma_start(
                    out=b_rep,
                    in_=bass.AP(
                        tensor=b_in.tensor, offset=sup * OSUP, ap=[[0, 128], [1, OSUP]]
                    ),
                )
                U = []
                for k in range(KT):
                    u = upool.tile([128, OSUP], F16, tag=f"u{k}", name=f"u{k}")
                    if k % 2 == 0:
                        # SWDGE casting DMA straight from DRAM
                        nc.gpsimd.dma_start(out=u, in_=qT_v[k, :, osl])
                    else:
                        us = ustage.tile([128, OSUP], I32, tag="us", name="us")
                        hw_engines[(k // 2) % 2].dma_start(out=us, in_=qT_v[k, :, osl])
                        nc.gpsimd.tensor_copy(u, us)
                    U.append(u)
                return s_rep, b_rep, U

            def main_tile(sup, t, s_rep, b_rep, U):
                pss = [
                    mpsum.tile([128, 512], F32, tag=f"ps{nb}", name=f"ps{nb}")
                    for nb in range(NB)
                ]
                for k in range(KT):
                    lhs = xhT[:, k * T_LOC + t * CHUNK : k * T_LOC + (t + 1) * CHUNK]
                    for nb in range(NB):
                        nc.tensor.matmul(
                            pss[nb],
                            lhsT=lhs,
                            rhs=U[k][:, nb * 512 : (nb + 1) * 512],
                            start=(k == 0),
                            stop=(k == KT - 1),
                        )
                for nb in range(NB):
                    ob = sup * OSUP + nb * 512
                    ysb = ypool.tile([128, 512], F32, tag="ysb", name="ysb")
                    nc.vector.tensor_tensor(
                        ysb, pss[nb], s_rep[:, nb * 512 : (nb + 1) * 512],
                        op=mybir.AluOpType.mult,
                    )
                    nc.vector.tensor_tensor(
                        ysb, ysb, b_rep[:, nb * 512 : (nb + 1) * 512],
                        op=mybir.AluOpType.add,
                    )
                    hw_engines[nb % 2].dma_start(
                        out=y[t * CHUNK : (t + 1) * CHUNK, ob : ob + 512], in_=ysb
                    )

            s_rep, b_rep, U = load_super(0)

            FREE = ST * CHUNK  # 4096 free elements per chunk buffer
            for c in range(NCHUNK):
                A = fw.tile([128, FREE], F32, tag="A", name="A")
                B = fw.tile([128, FREE], F16, tag="B", name="B")
                hw_engines[c % 2].dma_start(
                    out=A, in_=xT_v[:, :, c * CHUNK : (c + 1) * CHUNK]
                )
                # 5 outer butterfly stages over s; 2 fused strided ops per stage
                # ping-pong: A(f32)->B, B->A16view? simpler: B->C alternation via
                # two fp16 tiles sharing tag rotation
                srcs = [A, B]
                C2 = fw.tile([128, FREE], F16, tag="C2", name="C2", bufs=1)
                order = [B, C2, B, C2, B]
                src = A
                for stage, h in enumerate((1, 2, 4, 8, 16)):
                    dst = order[stage]
                    run = h * CHUNK
                    sv = src.rearrange("p (g two r) -> p g two r", two=2, r=run)
                    dv = dst.rearrange("p (g two r) -> p g two r", two=2, r=run)
                    nc.vector.tensor_add(
                        dv[:, :, 0, :], sv[:, :, 0, :], sv[:, :, 1, :]
                    )
                    nc.vector.tensor_sub(
                        dv[:, :, 1, :], sv[:, :, 0, :], sv[:, :, 1, :]
                    )
                    src = dst
                # inner H128 fold on PE (h128 pre-scaled 1/64); evict to xhT
                SPF = 512 // CHUNK  # s-blocks per 512-wide matmul
                for f in range(FREE // 512):
                    fps = fpsum.tile([128, 512], F32, tag="fps", name="fps")
                    nc.tensor.matmul(
                        fps,
                        lhsT=h128_sb,
                        rhs=src[:, f * 512 : (f + 1) * 512],
                        start=True,
                        stop=True,
                    )
                    nc.scalar.copy(
                        xh_v[:, f * SPF : (f + 1) * SPF, c * CHUNK : (c + 1) * CHUNK],
                        fps,
                    )
                # interleave super-0 main matmuls for this token tile
                main_tile(0, c, s_rep, b_rep, U)

            for sup in range(1, NSUP):
                s_rep, b_rep, U = load_super(sup)
                for t in range(NCHUNK):
                    main_tile(sup, t, s_rep, b_rep, U)

    nc.compile()
    return nc


def _get_nc():
    if "nc" not in _cache:
        _cache["nc"] = _build_nc()
    return _cache["nc"]


def _h128_scaled():
    a = np.eye(128, dtype=np.float64)
    n, hh = 128, 1
    while hh < n:
        a = a.reshape(-1, n // (2 * hh), 2, hh, n)
        l = a[:, :, 0].copy()
        r = a[:, :, 1].copy()
        a[:, :, 0] = l + r
        a[:, :, 1] = l - r
        a = a.reshape(-1, n)
        hh *= 2
    return (a / 64.0).astype(np.float16)


def kernel(x, Q_tilde, s_tilde, bias):
    nc = _get_nc()
    h128 = _h128_scaled()
    qT = np.ascontiguousarray(Q_tilde.T).astype(np.int32)
    s_flat = np.ascontiguousarray(s_tilde.reshape(-1)).astype(np.float32)
    b_flat = np.ascontiguousarray(bias.reshape(-1)).astype(np.float32)
    in_maps = []
    for c in range(NCORES):
        xTc = np.ascontiguousarray(
            x[c * T_LOC : (c + 1) * T_LOC, :].T
        ).astype(np.float32)
        in_maps.append(
            {"xT": xTc, "qT": qT, "h128": h128, "s_in": s_flat, "b_in": b_flat}
        )
    res = bass_utils.run_bass_kernel_spmd(nc, in_maps, list(range(NCORES)))
    yf = np.concatenate([res.results[c]["y"] for c in range(NCORES)], axis=0)
    return yf.astype(np.float32)
